# revision 21
# baseline (speedup 1.0000x reference)
"""GATv2 layer kernel for Trainium2, sharded across 8 NeuronCores.

Computation (reference):
    Wh = h @ W.T                       [N, F]
    s1 = Wh @ a1, s2 = Wh @ a2         [N]
    e  = leaky_relu(s1[:,None] + s2[None,:], 0.2)
    attention = softmax(e * adj, dim=1)
    out = attention @ Wh               [N, F]

Sharding: rows (destination nodes) split across 8 cores, 1024 rows each.

Key restructure: softmax is invariant to a per-row positive scale, so scale
row i by c_i = exp(-s1_i). With leaky(v) = max(v, 0.2v) and the 0/1 mask:

    masked entry   -> exp(max(0.2*s2_j - 0.8*s1_i, s2_j))  = B_ij
    unmasked entry -> z_i = exp(-s1_i)

so numerator row i is  [(adj .* B) @ Whext]_i + z_i*(S - [adj @ Whext]_i)
with Whext = [Wh | 1], S = sum_j Whext_j. Per 128-source chunk the device
work is only:

    ts  : L = (m08_bcast + 0.2*s2_j) max s2_j   (DVE 4x / some on Pool)
    ACT : B = Exp(L)                            (batched over 8 chunks)
    tt  : Q = B .* adjT                         (DVE 2x, batched over 8)
    PE  : accQ[t] += Q^T @ whext[ci],  accD[t] += adjT^T @ whext[ci]

Everything small (Wh, s1, s2, z, S, broadcasts) is precomputed on the host
and pre-tiled so adj streams in 8 big DMAs and Whext in one. PSUM holds
only the 8 persistent accumulator banks. Final fixup per 128-row tile:
num = accQ + z*(S - accD), out = num[:, :128] / num[:, 128].
"""
import sys

for _p in ("/opt/trn_rl_repo", "/root/.axon_site/_ro/trn_rl_repo"):
    if _p not in sys.path:
        sys.path.insert(0, _p)

import numpy as np
import ml_dtypes
from contextlib import ExitStack

from concourse import bacc, tile, mybir
from concourse.bass_utils import run_bass_kernel_spmd

f32 = mybir.dt.float32
bf16 = mybir.dt.bfloat16
AL = mybir.AluOpType
AF = mybir.ActivationFunctionType

N = 8192
F = 128
NCORES = 8
RPC = N // NCORES          # rows per core = 1024
RT = RPC // 128            # row tiles per core = 8
NCI = N // 128             # column chunks = 64
G = 8                      # chunks per batch group
NG = NCI // G              # number of groups = 8
POOL_TS = 0                # Pool tensor ops measured ~13x slower than model

_CACHE = {}


def _build():
    nc = bacc.Bacc("TRN2", target_bir_lowering=False)

    adj_ext = nc.declare_dram_parameter("adjc", [128, NCI * RPC], bf16,
                                        isOutput=False)   # pre-tiled [128, 65536]
    whe_ext = nc.declare_dram_parameter("whe2", [128, NCI * (F + 1)], bf16,
                                        isOutput=False)   # pre-tiled [128, 8256]
    sibc_ext = nc.declare_dram_parameter("m08bc", [128, RPC], bf16,
                                         isOutput=False)
    sj2_ext = nc.declare_dram_parameter("sj02", [128, NCI], f32, isOutput=False)
    sj1_ext = nc.declare_dram_parameter("sj10", [128, NCI], f32, isOutput=False)
    zc_ext = nc.declare_dram_parameter("zc", [128, RT], f32, isOutput=False)
    sbc_ext = nc.declare_dram_parameter("sbc", [128, F + 1], f32,
                                        isOutput=False)
    out_ext = nc.declare_dram_parameter("out", [RPC, F], f32, isOutput=True)

    with tile.TileContext(nc) as tc, ExitStack() as ctx:
        const = ctx.enter_context(tc.tile_pool(name="const", bufs=1))
        psum = ctx.enter_context(tc.tile_pool(name="psum", bufs=4, space="PSUM"))
        adj_pool = ctx.enter_context(tc.tile_pool(name="adjp", bufs=3))
        upool = ctx.enter_context(tc.tile_pool(name="upool", bufs=2))
        bpool = ctx.enter_context(tc.tile_pool(name="bpool", bufs=2))
        qpool = ctx.enter_context(tc.tile_pool(name="qpool", bufs=2))
        outp = ctx.enter_context(tc.tile_pool(name="outp", bufs=2))

        # persistent PSUM accumulators: 4 banks accQ + 4 banks accD
        qbank = [psum.tile([128, 512], f32, tag="qb", name=f"qb{b}", bufs=4)
                 for b in range(4)]
        dbank = [psum.tile([128, 512], f32, tag="db", name=f"db{b}", bufs=4)
                 for b in range(4)]
        accQ = [qbank[t // 2][:, 256 * (t % 2):256 * (t % 2) + F + 1]
                for t in range(RT)]
        accD = [dbank[t // 2][:, 256 * (t % 2):256 * (t % 2) + F + 1]
                for t in range(RT)]

        # small constants needed first by the DVE pipeline
        m08bc = const.tile([128, RPC], bf16)
        nc.sync.dma_start(out=m08bc, in_=sibc_ext[:, :])
        sj02 = const.tile([128, NCI], f32)
        nc.sync.dma_start(out=sj02, in_=sj2_ext[:, :])
        sj10 = const.tile([128, NCI], f32)
        nc.sync.dma_start(out=sj10, in_=sj1_ext[:, :])

        # startup order: first adjacency half, first Whext tiles, second
        # adjacency half, remaining Whext — so both the DVE pipeline and the
        # PE can start as early as possible
        adj_tiles = [adj_pool.tile([128, G * RPC], bf16, tag="adjT",
                                   name=f"adjT{g}") for g in range(NG)]
        whe_sb = const.tile([128, NCI * (F + 1)], bf16)
        wsplit = 16 * (F + 1)
        hw = (G // 2) * RPC
        nc.sync.dma_start(out=adj_tiles[0][:, 0:hw], in_=adj_ext[:, 0:hw])
        nc.sync.dma_start(out=whe_sb[:, 0:wsplit], in_=whe_ext[:, 0:wsplit])
        nc.sync.dma_start(out=adj_tiles[0][:, hw:2 * hw],
                          in_=adj_ext[:, hw:2 * hw])
        nc.sync.dma_start(out=whe_sb[:, wsplit:], in_=whe_ext[:, wsplit:])

        # fixup constants (needed only at the very end)
        zc = const.tile([128, RT], f32)
        nc.sync.dma_start(out=zc, in_=zc_ext[:, :])
        Sbc = const.tile([128, F + 1], f32)
        nc.sync.dma_start(out=Sbc, in_=sbc_ext[:, :])

        def whext(ci):
            return whe_sb[:, (F + 1) * ci:(F + 1) * ci + F + 1]

        # main loop, groups of G=8 source chunks; DMA/exp/tt/matmuls run at
        # half-group (4-chunk) granularity for tighter pipelining
        H = G // 2
        for g in range(NG):
            u8 = upool.tile([128, G * RPC], bf16, tag="u", name=f"u{g}")
            at8 = adj_tiles[g]
            for half in range(2):
                j0 = H * half
                sl = slice(RPC * j0, RPC * (j0 + H))
                if g > 0:
                    nc.sync.dma_start(
                        out=at8[:, sl],
                        in_=adj_ext[:, G * RPC * g + RPC * j0:
                                    G * RPC * g + RPC * (j0 + H)])
                for j in range(j0, j0 + H):
                    ci = G * g + j
                    nc.vector.tensor_scalar(
                        out=u8[:, RPC * j:RPC * j + RPC],
                        in0=m08bc,
                        scalar1=sj02[:, ci:ci + 1],
                        scalar2=sj10[:, ci:ci + 1], op0=AL.add, op1=AL.max)
                B8 = bpool.tile([128, H * RPC], bf16, tag="B", bufs=6,
                                name=f"B{g}_{half}")
                nc.scalar.activation(out=B8, in_=u8[:, sl], func=AF.Exp)
                Q8 = qpool.tile([128, H * RPC], bf16, tag="Q", bufs=3,
                                name=f"Q{g}_{half}")
                nc.vector.tensor_tensor(out=Q8, in0=B8,
                                        in1=at8[:, sl], op=AL.mult)
                last = (g == NG - 1 and half == 1)
                if not last:
                    for j in range(j0, j0 + H):
                        cid = G * g + j
                        for t in range(RT):
                            nc.tensor.matmul(
                                accQ[t],
                                lhsT=Q8[:, RPC * (j - j0) + 128 * t:
                                        RPC * (j - j0) + 128 * t + 128],
                                rhs=whext(cid),
                                start=(cid == 0 and t % 2 == 0),
                                stop=False,
                                skip_group_check=True)
                    for j in range(j0, j0 + H):
                        cid = G * g + j
                        for t in range(RT):
                            nc.tensor.matmul(
                                accD[t],
                                lhsT=at8[:, RPC * j + 128 * t:
                                         RPC * j + 128 * t + 128],
                                rhs=whext(cid),
                                start=(cid == 0 and t % 2 == 0),
                                stop=False,
                                skip_group_check=True)
                else:
                    # final half: t-outer order so early banks stop first and
                    # the fixups overlap the PE tail
                    for t in range(RT):
                        for j in range(j0, j0 + H):
                            cid = G * g + j
                            nc.tensor.matmul(
                                accQ[t],
                                lhsT=Q8[:, RPC * (j - j0) + 128 * t:
                                        RPC * (j - j0) + 128 * t + 128],
                                rhs=whext(cid),
                                start=False,
                                stop=(cid == NCI - 1),
                                skip_group_check=True)
                        for j in range(j0, j0 + H):
                            cid = G * g + j
                            nc.tensor.matmul(
                                accD[t],
                                lhsT=at8[:, RPC * j + 128 * t:
                                         RPC * j + 128 * t + 128],
                                rhs=whext(cid),
                                start=False,
                                stop=(cid == NCI - 1),
                                skip_group_check=True)

        # fixup + output: num = accQ + z*(S - accD); out = num/den
        for t in range(RT):
            SmD = outp.tile([128, F + 1], f32, tag="smd", name=f"smd{t}")
            nc.vector.scalar_tensor_tensor(
                out=SmD, in0=accD[t], scalar=-1.0, in1=Sbc,
                op0=AL.mult, op1=AL.add)
            num = outp.tile([128, F + 1], f32, tag="num", name=f"num{t}")
            nc.vector.scalar_tensor_tensor(
                out=num, in0=SmD, scalar=zc[:, t:t + 1], in1=accQ[t],
                op0=AL.mult, op1=AL.add)
            rinv = outp.tile([128, 1], f32, tag="rinv", name=f"rinv{t}")
            nc.vector.reciprocal(rinv, num[:, F:F + 1])
            o_t = outp.tile([128, F], f32, tag="o", name=f"o{t}")
            nc.scalar.mul(o_t, num[:, 0:F], rinv[:, 0:1])
            nc.sync.dma_start(out=out_ext[128 * t:128 * t + 128, :], in_=o_t)

    nc.compile()
    return nc


def _get_nc():
    if "nc" not in _CACHE:
        _CACHE["nc"] = _build()
    return _CACHE["nc"]


def kernel(h, adj, W, a, _trace=False, _trace_kwargs=None):
    h = np.asarray(h, dtype=np.float32)
    adj = np.asarray(adj, dtype=np.float32)
    W = np.asarray(W, dtype=np.float32)
    a = np.asarray(a, dtype=np.float32)
    bf = ml_dtypes.bfloat16

    # host precompute (all O(N*F) or smaller)
    Wh = h.astype(np.float64) @ W.T.astype(np.float64)       # [N, F]
    a1 = a[0, :F].astype(np.float64)
    a2 = a[0, F:].astype(np.float64)
    s1 = Wh @ a1                                             # [N]
    s2 = Wh @ a2                                             # [N]

    whext = np.concatenate(
        [Wh, np.ones((N, 1))], axis=1).astype(bf)            # [N, 129]
    S = whext.astype(np.float64).sum(axis=0)                 # [129]
    sbc = np.ascontiguousarray(
        np.broadcast_to(S.astype(np.float32), (128, F + 1)))
    # pre-tiled Whext: [128, 64*129], tile ci at cols 129*ci..129*ci+129
    whe2 = np.ascontiguousarray(
        whext.reshape(NCI, 128, F + 1).transpose(1, 0, 2).reshape(
            128, NCI * (F + 1)))

    sj02 = np.ascontiguousarray(
        (0.2 * s2).astype(np.float32).reshape(NCI, 128).T)   # [128, 64]
    sj10 = np.ascontiguousarray(
        s2.astype(np.float32).reshape(NCI, 128).T)           # [128, 64]

    m08 = (-0.8 * s1).astype(bf)                             # [N] bf16
    # z = exp(-s1_eff), consistent with the bf16-rounded -0.8*s1 on device
    zv = np.exp(m08.astype(np.float64) / 0.8).astype(np.float32)

    adj_bf = adj.astype(bf)                                  # 0/1: lossless

    nc = _get_nc()
    in_maps = []
    for c in range(NCORES):
        r0 = c * RPC
        # pre-tiled adjT block: [128, 64*1024],
        # [p, 1024*ci + i] = adj[r0+i, 128*ci+p]
        blk = adj_bf[r0:r0 + RPC, :]
        adjc = np.ascontiguousarray(
            blk.reshape(RPC, NCI, 128).transpose(2, 1, 0).reshape(
                128, NCI * RPC))
        mb = np.broadcast_to(m08[r0:r0 + RPC][None, :], (128, RPC))
        in_maps.append({
            "adjc": adjc,
            "whe2": whe2,
            "m08bc": np.ascontiguousarray(mb),
            "sj02": sj02,
            "sj10": sj10,
            "zc": np.ascontiguousarray(
                zv[r0:r0 + RPC].reshape(RT, 128).T),
            "sbc": sbc,
        })
    kw = {}
    if _trace:
        kw["trace"] = True
        kw.update(_trace_kwargs or {})
    res = run_bass_kernel_spmd(nc, in_maps, core_ids=list(range(NCORES)), **kw)
    out = np.concatenate([res.results[c]["out"] for c in range(NCORES)], axis=0)
    if _trace:
        return out, res
    return out
